# revision 4
# baseline (speedup 1.0000x reference)
"""Trainium2 Bass kernel for the retrieval-kNN problem (B=512, C=1000, D=512, K=10).

Math (equivalent to the reference, with the softmax rewritten unnormalized):
  rd0 = sqrt(cls_num * ex2^2 - ex1^2)             [C, D]
  E   = exp(-rd0)   (softmax numerator; the rowmax shift cancels exactly)
  S_c = sum_d E[c, d]                              (softmax denominator)
  simi[b, c] = (sum_d x2[b,d] E[c,d] - 2 x[b,d] (p E)[c,d] + (p^2 E)[c])/S_c
  topk-10 smallest simi per row -> conf = sum(vals)/vals, predict = label[argmin]

Sharding: classes are split across the 8 cores (125 each); every core computes
its distance-column slab for all 512 batch rows, an AllToAll redistributes to
batch-sharded [64, 1000] slabs, and each core finishes top-k locally.

All device arrays are fed pre-transposed ([D, C_loc]) so the contraction dim
sits on partitions and no on-chip transposes are needed.
"""
import sys

for _p in ("/opt/trn_rl_repo",):
    if _p not in sys.path:
        sys.path.insert(0, _p)

import numpy as np

B, C, D, K = 512, 1000, 512, 10
NCORES = 8
BL = B // NCORES   # 64 batch rows per core
CL = C // NCORES   # 125 classes per core
DT = D // 128      # 4 d-tiles

_CACHE = {}


class _SplitDrainTileContext:
    """Deferred import wrapper; see _make_tc."""


def _make_tc(nc):
    import concourse.mybir as mybir
    from concourse import tile
    from concourse.vector_clock import ScopedClock

    class SplitDrainTileContext(tile.TileContext):
        # The stock tail Drain carries every outstanding sem wait; this
        # walrus build rejects ctrl instructions with more than a couple of
        # sync waits. Keep one wait on the drain, put the rest on SP nops.
        def _drain_and_barrier(self, tick_clock, wait_clock):
            nc = self.nc
            drain_inst = nc.sync.drain()
            wait_clock.add_sem_waits(
                drain_inst.ins, ScopedClock({None: tick_clock.global_clock})
            )
            si = drain_inst.ins.sync_info
            waits = list(si.on_wait) if si and si.on_wait else []
            if len(waits) > 1:
                assert self.sems is not None
                name_to_sem = {s.name: s for s in self.sems.allocated().values()}
                drain_inst.ins.sync_info = mybir.SyncInfo(
                    on_wait=[waits[0]],
                    on_update=list(si.on_update) if si.on_update else [],
                )
                for w in waits[1:]:
                    nc.sync.nop()._wait_ge(name_to_sem[w.ant_name], w.wait_value)
            nc.all_engine_barrier()
            popped = nc._tile_sem_poison_stack.pop()
            assert popped is self._sem_poison
            nc.clear_and_free_semaphores(list(self.sems.allocated().values()))
            nc.all_engine_barrier()

    return SplitDrainTileContext(nc)


def _split_sync_waits(nc, limit=1):
    """Walrus rejects instructions carrying more than a couple of sync waits.

    For any instruction with more than `limit` waits, move the excess onto
    NoOps inserted just before it on the same engine (equivalent: the engine
    executes sequentially, so waits on a preceding NoOp gate the instruction
    the same way).
    """
    import concourse.mybir as mybir

    ctr = [0]
    for fn in nc.m.functions:
        for bb in fn.blocks:
            insts = bb.instructions
            i = 0
            while i < len(insts):
                inst = insts[i]
                si = inst.sync_info
                waits = list(si.on_wait) if si and si.on_wait else []
                if len(waits) > limit:
                    keep = waits[-limit:]
                    excess = waits[:-limit]
                    inst.sync_info = mybir.SyncInfo(
                        on_wait=keep,
                        on_update=list(si.on_update) if si.on_update else [],
                    )
                    nops = []
                    for j in range(0, len(excess), limit):
                        nop = mybir.InstNoOp(
                            name=f"I-splitw-{ctr[0]}", ins=[], outs=[])
                        ctr[0] += 1
                        nop.engine = inst.engine
                        nop.sync_info = mybir.SyncInfo(
                            on_wait=excess[j:j + limit], on_update=[])
                        nops.append(nop)
                    insts[i:i] = nops
                    i += len(nops)
                i += 1


def build_program():
    """Build the Bass program (same SPMD program for all 8 cores)."""
    import concourse.bass as bass
    import concourse.mybir as mybir

    dt = mybir.dt
    F32 = dt.float32
    Alu = mybir.AluOpType
    Act = mybir.ActivationFunctionType

    nc = bass.Bass("TRN2", target_bir_lowering=False, debug=False,
                   num_devices=NCORES)

    # Inputs (per-core): transposed slabs + replicated xT / rows.
    xT_d = nc.dram_tensor("xT", [D, B], F32, kind="ExternalInput").ap()
    pT_d = nc.dram_tensor("pT", [D, CL], F32, kind="ExternalInput").ap()
    e2T_d = nc.dram_tensor("e2T", [D, CL], F32, kind="ExternalInput").ap()
    e1T_d = nc.dram_tensor("e1T", [D, CL], F32, kind="ExternalInput").ap()
    nrow_d = nc.dram_tensor("nrow", [1, CL], F32, kind="ExternalInput").ap()
    label_d = nc.dram_tensor("labelr", [1, C], F32, kind="ExternalInput").ap()
    iota_d = nc.dram_tensor("iotar", [1, C], F32, kind="ExternalInput").ap()

    conf_d = nc.dram_tensor("conf", [BL, K], F32, kind="ExternalOutput").ap()
    pred_d = nc.dram_tensor("pred", [BL, 1], dt.int32, kind="ExternalOutput").ap()

    tc = _make_tc(nc)
    with tc:
        with tc.tile_pool(name="sbuf", bufs=1) as pool, \
             tc.tile_pool(name="psum", bufs=1, space="PSUM") as psum, \
             tc.tile_pool(name="dram", bufs=1, space="DRAM") as dram:

            # ---- loads ----
            xT = []
            x2T = []
            pT, e2T, e1T = [], [], []
            for t in range(DT):
                xt = pool.tile([128, B], F32, tag=f"xT{t}")
                nc.sync.dma_start(xt[:], xT_d[t * 128:(t + 1) * 128, :])
                xT.append(xt)
                for lst, src, nm in ((pT, pT_d, "p"), (e2T, e2T_d, "e2"),
                                     (e1T, e1T_d, "e1")):
                    tl = pool.tile([128, CL], F32, tag=f"{nm}T{t}")
                    nc.sync.dma_start(tl[:], src[t * 128:(t + 1) * 128, :])
                    lst.append(tl)
            nB = pool.tile([128, CL], F32, tag="nB")
            nc.sync.dma_start(nB[:], nrow_d.to_broadcast([128, CL]))
            labelB = pool.tile([BL, C], F32, tag="labelB")
            nc.sync.dma_start(labelB[:], label_d.to_broadcast([BL, C]))
            iotaB = pool.tile([BL, C], F32, tag="iotaB")
            nc.sync.dma_start(iotaB[:], iota_d.to_broadcast([BL, C]))

            ones_col = pool.tile([128, 1], F32, tag="ones_col")
            nc.vector.memset(ones_col[:], 1.0)
            ones_row = pool.tile([1, 128], F32, tag="ones_row")
            nc.vector.memset(ones_row[:], 1.0)

            # ---- per-class weights E, W2=-2pE, PPE=p^2 E; S and const sums ----
            S_ps = psum.tile([1, CL], F32, tag="S")
            cst_ps = psum.tile([1, CL], F32, tag="cst")
            E, W2 = [], []
            for t in range(DT):
                sq2 = pool.tile([128, CL], F32, tag=f"sq2_{t}")
                nc.scalar.activation(sq2[:], e2T[t][:], Act.Square)
                sq1 = pool.tile([128, CL], F32, tag=f"sq1_{t}")
                nc.scalar.activation(sq1[:], e1T[t][:], Act.Square)
                a = pool.tile([128, CL], F32, tag=f"a_{t}")
                nc.vector.tensor_mul(a[:], sq2[:], nB[:])
                u = pool.tile([128, CL], F32, tag=f"u_{t}")
                nc.vector.tensor_sub(u[:], a[:], sq1[:])
                rd0 = pool.tile([128, CL], F32, tag=f"rd0_{t}")
                nc.scalar.activation(rd0[:], u[:], Act.Sqrt)
                e = pool.tile([128, CL], F32, tag=f"E_{t}")
                nc.scalar.activation(e[:], rd0[:], Act.Exp, scale=-1.0)
                E.append(e)
                w2 = pool.tile([128, CL], F32, tag=f"W2_{t}")
                nc.vector.scalar_tensor_tensor(w2[:], pT[t][:], -2.0, e[:],
                                               op0=Alu.mult, op1=Alu.mult)
                W2.append(w2)
                ppe = pool.tile([128, CL], F32, tag=f"PPE_{t}")
                nc.vector.scalar_tensor_tensor(ppe[:], pT[t][:], -0.5, w2[:],
                                               op0=Alu.mult, op1=Alu.mult)
                # x^2 in transposed layout
                x2t = pool.tile([128, B], F32, tag=f"x2T{t}")
                nc.scalar.activation(x2t[:], xT[t][:], Act.Square)
                x2T.append(x2t)

                nc.tensor.matmul(S_ps[:], lhsT=ones_col[:], rhs=e[:],
                                 start=(t == 0), stop=(t == DT - 1))
                nc.tensor.matmul(cst_ps[:], lhsT=ones_col[:], rhs=ppe[:],
                                 start=(t == 0), stop=(t == DT - 1))

            # const row to SBUF; nr = -1/S broadcast to all partitions
            cst_sb = pool.tile([1, CL], F32, tag="cst_sb")
            nc.scalar.copy(cst_sb[:], cst_ps[:])
            rS = pool.tile([1, CL], F32, tag="rS")
            nc.vector.reciprocal(rS[:], S_ps[:])
            nr = pool.tile([1, CL], F32, tag="nr")
            nc.vector.tensor_scalar_mul(nr[:], rS[:], -1.0)
            nrB_ps = psum.tile([128, CL], F32, tag="nrB")
            nc.tensor.matmul(nrB_ps[:], lhsT=ones_row[:], rhs=nr[:],
                             start=True, stop=True)
            nrB = pool.tile([128, CL], F32, tag="nrB_sb")
            nc.scalar.copy(nrB[:], nrB_ps[:])

            # ---- distance slab: neg_simi[b, c] for all 512 b, my 125 c ----
            a2a_in = dram.tile([B, CL], F32, tag="a2a_in")
            for m in range(4):
                usim = psum.tile([128, CL], F32, tag=f"usim{m}")
                sl = slice(m * 128, (m + 1) * 128)
                for t in range(DT):
                    nc.tensor.matmul(usim[:], lhsT=x2T[t][:, sl], rhs=E[t][:],
                                     start=(t == 0), stop=False)
                for t in range(DT):
                    nc.tensor.matmul(usim[:], lhsT=xT[t][:, sl], rhs=W2[t][:],
                                     start=False, stop=False)
                nc.tensor.matmul(usim[:], lhsT=ones_row[:], rhs=cst_sb[:],
                                 start=False, stop=True)
                neg = pool.tile([128, CL], F32, tag=f"neg{m}")
                nc.vector.tensor_mul(neg[:], usim[:], nrB[:])
                nc.sync.dma_start(a2a_in[sl, :], neg[:])

            # ---- exchange: chunk j of my columns -> core j's rows ----
            import concourse.mybir as mybir2
            a2a_out = dram.tile([B, CL], F32, tag="a2a_out")
            nc.gpsimd.collective_compute(
                "AllToAll",
                mybir2.AluOpType.bypass,
                replica_groups=[list(range(NCORES))],
                ins=[a2a_in[:]],
                outs=[a2a_out[:]],
            )

            # ---- regather to [64, 1000] (my rows, all classes) ----
            nsimi = pool.tile([BL, C], F32, tag="nsimi")
            for g in range(NCORES):
                nc.sync.dma_start(nsimi[:, g * CL:(g + 1) * CL],
                                  a2a_out[g * BL:(g + 1) * BL, :])

            # ---- top-10 smallest distances (largest neg_simi) ----
            v1 = pool.tile([BL, 8], F32, tag="v1")
            i1 = pool.tile([BL, 8], dt.uint32, tag="i1")
            nc.vector.max(v1[:], nsimi[:])
            nc.vector.max_index(i1[:], v1[:], nsimi[:])
            ns2 = pool.tile([BL, C], F32, tag="ns2")
            nc.vector.match_replace(ns2[:], v1[:], nsimi[:], -3.0e38)
            v2 = pool.tile([BL, 8], F32, tag="v2")
            nc.vector.max(v2[:], ns2[:])

            posv = pool.tile([BL, K], F32, tag="posv")
            nc.vector.tensor_scalar_mul(posv[:, :8], v1[:], -1.0)
            nc.vector.tensor_scalar_mul(posv[:, 8:K], v2[:, :K - 8], -1.0)
            s10 = pool.tile([BL, 1], F32, tag="s10")
            nc.vector.tensor_reduce(s10[:], posv[:], axis=mybir.AxisListType.X,
                                    op=Alu.add)
            rv = pool.tile([BL, K], F32, tag="rv")
            nc.vector.reciprocal(rv[:], posv[:])
            conf = pool.tile([BL, K], F32, tag="conf")
            nc.vector.tensor_scalar_mul(conf[:], rv[:], s10[:])
            nc.sync.dma_start(conf_d[:], conf[:])

            # ---- predict = label[argmin dist] (= index of max neg_simi) ----
            ansf = pool.tile([BL, 1], F32, tag="ansf")
            nc.vector.tensor_copy(ansf[:], i1[:, 0:1])
            eq = pool.tile([BL, C], F32, tag="eq")
            nc.vector.tensor_single_scalar(eq[:], iotaB[:], ansf[:],
                                           op=Alu.is_equal)
            lm = pool.tile([BL, C], F32, tag="lm")
            nc.vector.tensor_mul(lm[:], eq[:], labelB[:])
            predf = pool.tile([BL, 1], F32, tag="predf")
            nc.vector.tensor_reduce(predf[:], lm[:], axis=mybir.AxisListType.X,
                                    op=Alu.max)
            predi = pool.tile([BL, 1], dt.int32, tag="predi")
            nc.vector.tensor_copy(predi[:], predf[:])
            nc.sync.dma_start(pred_d[:], predi[:])

    _split_sync_waits(nc)
    return nc


def make_in_maps(x, protos, ex2, ex1, cls_num, proto_label):
    x = np.asarray(x, dtype=np.float32)
    protos = np.asarray(protos, dtype=np.float32)
    ex2 = np.asarray(ex2, dtype=np.float32)
    ex1 = np.asarray(ex1, dtype=np.float32)
    cls_f = np.asarray(cls_num).astype(np.float32)
    label_f = np.asarray(proto_label).astype(np.float32)[None, :]
    iota_f = np.arange(C, dtype=np.float32)[None, :]
    xT = np.ascontiguousarray(x.T)
    in_maps = []
    for g in range(NCORES):
        sl = slice(g * CL, (g + 1) * CL)
        in_maps.append({
            "xT": xT,
            "pT": np.ascontiguousarray(protos[sl].T),
            "e2T": np.ascontiguousarray(ex2[sl].T),
            "e1T": np.ascontiguousarray(ex1[sl].T),
            "nrow": np.ascontiguousarray(cls_f[sl][None, :]),
            "labelr": label_f,
            "iotar": iota_f,
        })
    return in_maps


def assemble(results):
    conf = np.concatenate([results[g]["conf"] for g in range(NCORES)], axis=0)
    pred = np.concatenate([results[g]["pred"][:, 0] for g in range(NCORES)],
                          axis=0).astype(np.int32)
    return pred, conf


def kernel(x, protos, ex2, ex1, cls_num, proto_label, k_nearest=K,
           _trace=False, _tmpdir=None):
    from concourse.bass_utils import run_bass_kernel_spmd

    if "nc" not in _CACHE:
        _CACHE["nc"] = build_program()
    nc = _CACHE["nc"]
    in_maps = make_in_maps(x, protos, ex2, ex1, cls_num, proto_label)
    res = run_bass_kernel_spmd(nc, in_maps, core_ids=list(range(NCORES)),
                               trace=_trace, tmpdir=_tmpdir)
    _CACHE["last_res"] = res
    return assemble(res.results)


# revision 15
# speedup vs baseline: 1.0218x; 1.0218x over previous
"""Trainium2 Bass kernel for the retrieval-kNN problem (B=512, C=1000, D=512, K=10).

Math (equivalent to the reference, with the softmax rewritten unnormalized —
the rowmax shift inside the reference softmax cancels exactly):
  E   = exp(-sqrt(cls_num * ex2^2 - ex1^2))        [C, D] softmax numerator
  S_c = sum_d E[c, d]                               softmax denominator
  usim[b,c] = sum_d x2[b,d] E[c,d] + x[b,d] W2[c,d] + 0.25 * cst4[c]
      with W2 = -2 p E  (pm2 = -2p fed from host),
           cst4 = sum_d pm2^2 E = 4 sum_d p^2 E
  simi = usim / S_c
  top-10 smallest simi per row -> conf = sum(vals)/vals,
  predict = proto_label[argmin simi]

Sharding: pure data parallel over the batch (64 rows per core); the per-class
matrices are replicated. Everything is fed host-transposed ([D, C]) so the
contraction dim d sits on partitions: no on-chip transposes, and the softmax
denominator falls out of the main matmul via an extra all-ones lhsT column.
The class dim is split 512/488 into left/right tiles: the left elementwise
chain runs on DVE, the right mostly on GpSimd, sqrt/exp on ScalarE (batched
per function so the activation table loads only twice). No collectives.
"""
import sys

for _p in ("/opt/trn_rl_repo",):
    if _p not in sys.path:
        sys.path.insert(0, _p)

import numpy as np

B, C, D, K = 512, 1000, 512, 10
NCORES = 8
BL = B // NCORES   # 64 batch rows per core
DT = D // 128      # 4 d-tiles
SP = 512           # left/right class-column split (PSUM bank aligned)
CW = (SP, C - SP)  # 512, 488
BIG = float(1 << 24)

_CACHE = {}


def _make_tc(nc):
    import concourse.mybir as mybir
    from concourse import tile
    from concourse.vector_clock import ScopedClock

    class SplitDrainTileContext(tile.TileContext):
        # The stock tail Drain carries every outstanding sem wait; this
        # walrus build rejects instructions with more than one sync wait.
        # Keep one wait on the drain, put the rest on SP nops.
        def _drain_and_barrier(self, tick_clock, wait_clock):
            nc = self.nc
            drain_inst = nc.sync.drain()
            wait_clock.add_sem_waits(
                drain_inst.ins, ScopedClock({None: tick_clock.global_clock})
            )
            si = drain_inst.ins.sync_info
            waits = list(si.on_wait) if si and si.on_wait else []
            if len(waits) > 1:
                assert self.sems is not None
                name_to_sem = {s.name: s for s in self.sems.allocated().values()}
                drain_inst.ins.sync_info = mybir.SyncInfo(
                    on_wait=[waits[0]],
                    on_update=list(si.on_update) if si.on_update else [],
                )
                for w in waits[1:]:
                    nc.sync.nop()._wait_ge(name_to_sem[w.ant_name], w.wait_value)
            nc.all_engine_barrier()
            popped = nc._tile_sem_poison_stack.pop()
            assert popped is self._sem_poison
            nc.clear_and_free_semaphores(list(self.sems.allocated().values()))
            nc.all_engine_barrier()

    return SplitDrainTileContext(nc)


def _split_sync_waits(nc, limit=1):
    """Move excess sem waits onto NoOps inserted just before the owning
    instruction on the same engine (walrus rejects multi-wait instructions)."""
    import concourse.mybir as mybir

    ctr = [0]
    for fn in nc.m.functions:
        for bb in fn.blocks:
            insts = bb.instructions
            i = 0
            while i < len(insts):
                inst = insts[i]
                si = inst.sync_info
                waits = list(si.on_wait) if si and si.on_wait else []
                if len(waits) > limit:
                    keep = waits[-limit:]
                    excess = waits[:-limit]
                    inst.sync_info = mybir.SyncInfo(
                        on_wait=keep,
                        on_update=list(si.on_update) if si.on_update else [],
                    )
                    nops = []
                    for j in range(0, len(excess), limit):
                        nop = mybir.InstNoOp(
                            name=f"I-splitw-{ctr[0]}", ins=[], outs=[])
                        ctr[0] += 1
                        nop.engine = inst.engine
                        nop.sync_info = mybir.SyncInfo(
                            on_wait=excess[j:j + limit], on_update=[])
                        nops.append(nop)
                    insts[i:i] = nops
                    i += len(nops)
                i += 1


def build_program(split_waits=True):
    """Build the Bass program (same SPMD program for all 8 cores)."""
    import concourse.bass as bass
    import concourse.mybir as mybir

    dt = mybir.dt
    F32 = dt.float32
    Alu = mybir.AluOpType
    Act = mybir.ActivationFunctionType
    X = mybir.AxisListType.X

    nc = bass.Bass("TRN2", target_bir_lowering=False, debug=False,
                   num_devices=NCORES)

    # Replicated transposed per-class matrices; per-core xT slab (64 cols).
    pm2T_d = nc.dram_tensor("pm2T", [D, C], F32, kind="ExternalInput").ap()
    e2T_d = nc.dram_tensor("e2T", [D, C], F32, kind="ExternalInput").ap()
    e1T_d = nc.dram_tensor("e1T", [D, C], F32, kind="ExternalInput").ap()
    xT_d = nc.dram_tensor("xT", [D, BL], F32, kind="ExternalInput").ap()
    nrow_d = nc.dram_tensor("nrow", [1, C], F32, kind="ExternalInput").ap()
    label_d = nc.dram_tensor("labelr", [1, C], F32, kind="ExternalInput").ap()

    conf_d = nc.dram_tensor("conf", [BL, K], F32, kind="ExternalOutput").ap()
    pred_d = nc.dram_tensor("pred", [BL, 1], dt.int32, kind="ExternalOutput").ap()

    tc = _make_tc(nc)
    with tc:
        with tc.tile_pool(name="sbuf", bufs=1) as pool, \
             tc.tile_pool(name="psum", bufs=1, space="PSUM") as psum, \
             tc.tile_pool(name="dram", bufs=1, space="DRAM") as dram:

            # ---- loads; class columns split into left [0:512) / right ----
            # e2/e1/pm2 tiles are reused in place: e2 -> u -> E,
            # e1 -> rd0 -> W2, pm2 -> PPE.
            e2, e1, pm2 = [[], []], [[], []], [[], []]
            xT = []
            for t in range(DT):
                rows = slice(t * 128, (t + 1) * 128)
                for h in range(2):
                    cols = slice(h * SP, h * SP + CW[h])
                    for lst, src, nm in ((e2, e2T_d, "e2"), (e1, e1T_d, "e1"),
                                         (pm2, pm2T_d, "pm2")):
                        tl = pool.tile([128, CW[h]], F32, tag=f"{nm}_{t}_{h}")
                        nc.sync.dma_start(tl[:], src[rows, cols])
                        lst[h].append(tl)
                xt = pool.tile([128, BL], F32, tag=f"xT{t}")
                nc.sync.dma_start(xt[:], xT_d[rows, :])
                xT.append(xt)
            nB = pool.tile([128, C], F32, tag="nB")
            nc.sync.dma_start(nB[:], nrow_d.to_broadcast([128, C]))
            labelB = pool.tile([BL, C], F32, tag="labelB")
            nc.sync.dma_start(labelB[:], label_d.to_broadcast([BL, C]))

            # x2/ones lhsT: cols 0-63 x^2, col 64 ones -> S row;
            # x lhsT: cols 0-63 x, col 64 zero.
            x2a, xa = [], []
            for t in range(DT):
                x2 = pool.tile([128, BL + 1], F32, tag=f"x2a{t}")
                nc.vector.tensor_mul(x2[:, :BL], xT[t][:], xT[t][:])
                nc.vector.memset(x2[:, BL:], 1.0)
                x2a.append(x2)
                xz = pool.tile([128, BL + 1], F32, tag=f"xa{t}")
                nc.vector.tensor_copy(xz[:, :BL], xT[t][:])
                nc.vector.memset(xz[:, BL:], 0.0)
                xa.append(xz)
            # rank-1 const row: 0.25 over batch cols (cst4 = 4*cst), 0 at S
            q_row = pool.tile([1, BL + 1], F32, tag="q_row")
            nc.vector.memset(q_row[:, :BL], 0.25)
            nc.vector.memset(q_row[:, BL:], 0.0)
            ones_col = pool.tile([128, 1], F32, tag="ones_col")
            nc.vector.memset(ones_col[:], 1.0)

            # engine per (half, op): left chain on DVE; right squares/W2/PPE
            # on GpSimd, right mul/sub on DVE (balance).
            V, G = nc.vector, nc.gpsimd

            def TT(eng, out, a, b, op=Alu.mult):
                eng.tensor_tensor(out, a, b, op=op)

            # ---- E-chain (in place) ----
            for t in range(DT):
                for h, sq_eng, ms_eng in ((0, V, V), (1, G, V)):
                    cols = slice(h * SP, h * SP + CW[h])
                    TT(sq_eng, e2[h][t][:], e2[h][t][:], e2[h][t][:])
                    TT(ms_eng, e2[h][t][:], e2[h][t][:], nB[:, cols])
                    TT(sq_eng, e1[h][t][:], e1[h][t][:], e1[h][t][:])
                    TT(ms_eng, e2[h][t][:], e2[h][t][:], e1[h][t][:],
                       op=Alu.subtract)
            for t in range(DT):
                for h in range(2):
                    nc.scalar.activation(e1[h][t][:], e2[h][t][:], Act.Sqrt)
            E, W2 = e2, e1
            for t in range(DT):
                for h in range(2):
                    nc.scalar.activation(E[h][t][:], e1[h][t][:], Act.Exp,
                                         scale=-1.0)
            # W2 = pm2*E ; PPE4 = pm2*W2 (=4 p^2 E) accumulated into pm2[h][0]
            for t in range(DT):
                for h, eng in ((0, V), (1, G)):
                    TT(eng, W2[h][t][:], pm2[h][t][:], E[h][t][:])
                    TT(eng, pm2[h][t][:], pm2[h][t][:], W2[h][t][:])
            for h, eng in ((0, V), (1, G)):
                TT(eng, pm2[h][0][:], pm2[h][0][:], pm2[h][1][:], op=Alu.add)
                TT(eng, pm2[h][2][:], pm2[h][2][:], pm2[h][3][:], op=Alu.add)
                TT(eng, pm2[h][0][:], pm2[h][0][:], pm2[h][2][:], op=Alu.add)

            # ---- cst4 row via thin ones-matmul; usim[65, 1024] ----
            cst_ps = psum.tile([1, 1024], F32, tag="cst_ps")
            cst = pool.tile([1, C], F32, tag="cst")
            usim = psum.tile([BL + 1, 1024], F32, tag="usim")
            for h in range(2):
                pcols = slice(h * SP, h * SP + CW[h])
                nc.tensor.matmul(cst_ps[:, pcols], lhsT=ones_col[:],
                                 rhs=pm2[h][0][:], start=True, stop=True)
                nc.vector.tensor_copy(cst[:, pcols], cst_ps[:, pcols])
                for t in range(DT):
                    nc.tensor.matmul(usim[:, pcols], lhsT=x2a[t][:],
                                     rhs=E[h][t][:],
                                     start=(t == 0), stop=False)
                for t in range(DT):
                    nc.tensor.matmul(usim[:, pcols], lhsT=xa[t][:],
                                     rhs=W2[h][t][:],
                                     start=False, stop=False)
                nc.tensor.matmul(usim[:, pcols], lhsT=q_row[:],
                                 rhs=cst[:, pcols], start=False, stop=True)

            # ---- neg_simi = usim * (-1/S) ----
            nnr = pool.tile([1, C], F32, tag="nnr")
            nc.vector.reciprocal(nnr[:], usim[BL:BL + 1, :C])
            nc.vector.tensor_scalar_mul(nnr[:], nnr[:], -1.0)
            nnr_dr = dram.tile([1, C], F32, tag="nnr_dr")
            nc.sync.dma_start(nnr_dr[:], nnr[:])
            nrB = pool.tile([BL, C], F32, tag="nrB_sb")
            nc.sync.dma_start(nrB[:], nnr_dr[:].to_broadcast([BL, C]))
            nsimi = pool.tile([BL, C], F32, tag="nsimi")
            nc.vector.tensor_mul(nsimi[:, :SP], usim[:BL, :SP], nrB[:, :SP])
            nc.vector.tensor_mul(nsimi[:, SP:], usim[:BL, SP:C], nrB[:, SP:])

            # ---- top-10 smallest distances (largest neg_simi) ----
            v1 = pool.tile([BL, 8], F32, tag="v1")
            nc.vector.max(v1[:], nsimi[:])
            ns2 = pool.tile([BL, C], F32, tag="ns2")
            nc.vector.match_replace(ns2[:], v1[:], nsimi[:], -3.0e38)
            v2 = pool.tile([BL, 8], F32, tag="v2")
            nc.vector.max(v2[:], ns2[:])

            posv = pool.tile([BL, K], F32, tag="posv")
            nc.vector.tensor_scalar_mul(posv[:, :8], v1[:], -1.0)
            nc.vector.tensor_scalar_mul(posv[:, 8:K], v2[:, :K - 8], -1.0)
            s10 = pool.tile([BL, 1], F32, tag="s10")
            nc.vector.tensor_reduce(s10[:], posv[:], axis=X, op=Alu.add)
            rv = pool.tile([BL, K], F32, tag="rv")
            nc.vector.reciprocal(rv[:], posv[:])
            conf = pool.tile([BL, K], F32, tag="conf")
            nc.vector.tensor_scalar_mul(conf[:], rv[:], s10[:])
            nc.sync.dma_start(conf_d[:], conf[:])

            # ---- predict: label of the (first) argmin distance ----
            # eq = (nsimi == min) ; masked = label - BIG*eq ; rowmin + BIG
            negbig = pool.tile([BL, C], F32, tag="negbig")
            nc.gpsimd.memset(negbig[:], -BIG)
            eq = pool.tile([BL, C], F32, tag="eq")
            nc.vector.tensor_single_scalar(eq[:], nsimi[:], v1[:, 0:1],
                                           op=Alu.is_equal)
            nc.gpsimd.tensor_mul(eq[:], eq[:], negbig[:])
            masked = pool.tile([BL, C], F32, tag="masked")
            nc.gpsimd.tensor_add(masked[:], eq[:], labelB[:])
            predf = pool.tile([BL, 1], F32, tag="predf")
            nc.vector.tensor_reduce(predf[:], masked[:], axis=X, op=Alu.min)
            predb = pool.tile([BL, 1], F32, tag="predb")
            nc.vector.tensor_scalar_add(predb[:], predf[:], BIG)
            predi = pool.tile([BL, 1], dt.int32, tag="predi")
            nc.vector.tensor_copy(predi[:], predb[:])
            nc.sync.dma_start(pred_d[:], predi[:])

    if split_waits:
        _split_sync_waits(nc)
    return nc


def make_in_maps(x, protos, ex2, ex1, cls_num, proto_label):
    x = np.asarray(x, dtype=np.float32)
    protos = np.asarray(protos, dtype=np.float32)
    ex2 = np.asarray(ex2, dtype=np.float32)
    ex1 = np.asarray(ex1, dtype=np.float32)
    cls_f = np.ascontiguousarray(np.asarray(cls_num).astype(np.float32)[None, :])
    label_f = np.ascontiguousarray(np.asarray(proto_label).astype(np.float32)[None, :])
    pm2T = np.ascontiguousarray((-2.0 * protos).T)
    e2T = np.ascontiguousarray(ex2.T)
    e1T = np.ascontiguousarray(ex1.T)
    xT = np.ascontiguousarray(x.T)
    in_maps = []
    for g in range(NCORES):
        in_maps.append({
            "pm2T": pm2T,
            "e2T": e2T,
            "e1T": e1T,
            "xT": np.ascontiguousarray(xT[:, g * BL:(g + 1) * BL]),
            "nrow": cls_f,
            "labelr": label_f,
        })
    return in_maps


def assemble(results):
    conf = np.concatenate([results[g]["conf"] for g in range(NCORES)], axis=0)
    pred = np.concatenate([results[g]["pred"][:, 0] for g in range(NCORES)],
                          axis=0).astype(np.int32)
    return pred, conf


def kernel(x, protos, ex2, ex1, cls_num, proto_label, k_nearest=K,
           _trace=False, _tmpdir=None):
    from concourse.bass_utils import run_bass_kernel_spmd

    if "nc" not in _CACHE:
        _CACHE["nc"] = build_program()
    nc = _CACHE["nc"]
    in_maps = make_in_maps(x, protos, ex2, ex1, cls_num, proto_label)
    res = run_bass_kernel_spmd(nc, in_maps, core_ids=list(range(NCORES)),
                               trace=_trace, tmpdir=_tmpdir)
    _CACHE["last_res"] = res
    return assemble(res.results)
